# revision 3
# baseline (speedup 1.0000x reference)
"""Trainium2 Bass kernel for the Deep OSTTP model.

Model (reference):
    z = x @ W_in.T + b_in                          # [2048, 2048]
    for i in 0..3:
        pre = z @ Wx[i].T
        h = 0
        repeat 8: h = tanh(h @ Wz[i].T + bz[i] + pre)
        z = h
    out = z @ W_head.T + b_head                    # [2048, 1024]

Sharding: data-parallel over the batch dim; each of the 8 NeuronCores gets a
[256, 2048] slice of x and replicated weights. No collectives.

On-chip layout: activations are kept transposed (zT: [H=2048 partitions-dim,
B=256 free]) so every linear layer is computed as y.T = W @ z.T with the
weight block (W.T tile) as the stationary matmul operand and the activation
k-tile as the moving operand.

Numerics: all matmuls run in float32r (fp32 with the low 12 mantissa bits
cleared) — at moving free-dim >= 256 the PE streams f32r at full (bf16) rate,
and on pre-rounded operands the matmul is exact, so the only error vs the
fp32 reference is the ~1.2e-4 input rounding. f32r operands MUST be
pre-rounded (unrounded bit patterns hard-fault the PE), so weights are
rounded on the host and activations are produced as f32r by ScalarE.

Wz as generated by the model init is exactly 0.5*I, which makes the
recurrence elementwise: h = tanh(0.5*h + c), c = pre + bz. kernel() verifies
this at runtime and uses the elementwise fast path (DVE add + ScalarE tanh,
fp32-exact); otherwise it falls back to a dense-Wz program.
"""

import sys

import numpy as np

try:
    import concourse  # noqa: F401
except ImportError:  # pragma: no cover
    sys.path.insert(0, "/opt/trn_rl_repo")

import concourse.tile as tile
from concourse import bacc, mybir
from concourse.bass_utils import run_bass_kernel_spmd

F32 = mybir.dt.float32
F32R = mybir.dt.float32r
TANH = mybir.ActivationFunctionType.Tanh
IDENT = mybir.ActivationFunctionType.Identity

NCORES = 8
H = 2048          # hidden width == input width
OUT = 1024        # head width
B = 2048 // NCORES  # per-core batch
KT = H // 128     # contraction tiles
NL = 4            # layers
NS = 8            # recurrence steps


def _round_f32r(x: np.ndarray) -> np.ndarray:
    """Round fp32 to f32r: clear low 12 mantissa bits, round-to-nearest-even."""
    b = np.ascontiguousarray(x, dtype=np.float32).view(np.uint32).copy()
    low = b & np.uint32(0xFFF)
    b &= np.uint32(0xFFFFF000)
    lsb = (b >> np.uint32(12)) & np.uint32(1)
    up = (low > 0x800) | ((low == 0x800) & (lsb == 1))
    b += up.astype(np.uint32) << np.uint32(12)
    return b.view(np.float32)


def _is_half_identity(wz: np.ndarray) -> bool:
    if wz.shape != (NL, H, H):
        return False
    diag = np.einsum("lii->li", wz)
    return bool((diag == 0.5).all()) and np.count_nonzero(wz) == NL * H


def _build(wz_diag: bool):
    nc = bacc.Bacc("TRN2", target_bir_lowering=False, debug=False,
                   num_devices=NCORES)

    xT_d = nc.dram_tensor("xT", [H, B], F32R, kind="ExternalInput")
    win_d = nc.dram_tensor("win", [H, H], F32R, kind="ExternalInput")
    wx_d = [nc.dram_tensor(f"wx{i}", [H, H], F32R, kind="ExternalInput")
            for i in range(NL)]
    whead_d = nc.dram_tensor("whead", [H, OUT], F32R, kind="ExternalInput")
    bin_d = nc.dram_tensor("bin", [H], F32, kind="ExternalInput")
    bz_d = nc.dram_tensor("bz", [NL, H], F32, kind="ExternalInput")
    bh_d = nc.dram_tensor("bh", [OUT], F32, kind="ExternalInput")
    o_d = nc.dram_tensor("o", [OUT, B], F32, kind="ExternalOutput")
    if not wz_diag:
        wz_d = [nc.dram_tensor(f"wz{i}", [H, H], F32R, kind="ExternalInput")
                for i in range(NL)]
        id_d = nc.dram_tensor("ident", [128, 128], F32R, kind="ExternalInput")

    with tile.TileContext(nc) as tc:
        from contextlib import ExitStack
        with ExitStack() as ctx:
            wpool = ctx.enter_context(tc.tile_pool(name="wpool", bufs=18))
            apool = ctx.enter_context(tc.tile_pool(name="apool", bufs=3))
            hpool = ctx.enter_context(tc.tile_pool(name="hpool", bufs=2))
            spool = ctx.enter_context(tc.tile_pool(name="spool", bufs=2))
            cpool = ctx.enter_context(tc.tile_pool(name="cpool", bufs=1))
            kpool = ctx.enter_context(tc.tile_pool(name="kpool", bufs=1))
            opool = ctx.enter_context(tc.tile_pool(name="opool", bufs=1))
            pspool = ctx.enter_context(
                tc.tile_pool(name="pspool", bufs=6, space="PSUM"))

            # constants
            bin_t = kpool.tile([128, KT], F32, tag="bin")
            nc.sync.dma_start(bin_t[:], bin_d.rearrange("(t p) -> p t", p=128))
            bz_t = kpool.tile([128, NL, KT], F32, tag="bz")
            nc.sync.dma_start(bz_t[:], bz_d.rearrange("l (t p) -> p l t", p=128))
            bh_t = kpool.tile([128, OUT // 128], F32, tag="bh")
            nc.sync.dma_start(bh_t[:], bh_d.rearrange("(t p) -> p t", p=128))
            if not wz_diag:
                id_t = kpool.tile([128, 128], F32R, tag="ident")
                nc.sync.dma_start(id_t[:], id_d[:])

            # activation input
            x_t = apool.tile([128, KT, B], F32R, tag="act")
            nc.sync.dma_start(x_t[:], xT_d.rearrange("(kt p) n -> p kt n", p=128))

            def phase(w_d, w_cols, ch, in_t, emit_out, extra_mm=None):
                """out.T[m] = sum_k (W.T)[k,m].T @ in[k]  (+ extra), by column
                halves of width ch so at most KT tiles of [128, ch] weights
                are resident at once."""
                nhalf = w_cols // ch
                w_r = w_d.rearrange("(kt p) (mh c) -> p kt mh c", p=128, c=ch)
                mm_per = ch // 128
                for mh in range(nhalf):
                    wts = []
                    for k in range(KT):
                        wt = wpool.tile([128, ch], F32R, tag="wt")
                        nc.sync.dma_start(wt[:], w_r[:, k, mh, :])
                        wts.append(wt)
                    for mm in range(mm_per):
                        m = mh * mm_per + mm
                        ps = pspool.tile([128, B], F32, tag="ps")
                        started = False
                        if extra_mm is not None:
                            extra_mm(m, ps)
                            started = True
                        for k in range(KT):
                            nc.tensor.matmul(
                                ps[:], wts[k][:, mm * 128:(mm + 1) * 128],
                                in_t[:, k, :],
                                start=(not started and k == 0),
                                stop=(k == KT - 1))
                        emit_out(m, ps)

            # ---- input projection: z1.T = W_in @ x.T + b_in
            z_t = apool.tile([128, KT, B], F32R, tag="act")

            def emit_z(m, ps):
                nc.scalar.activation(z_t[:, m, :], ps[:], IDENT,
                                     bias=bin_t[:, m:m + 1], scale=1.0)

            with nc.named_scope("zproj"):
                phase(win_d, H, 1024, x_t, emit_z)

            # ---- layers
            for i in range(NL):
                if wz_diag:
                    # c2 = 2*(pre + bz);  h <- tanh(0.5*(h + c2))
                    c2_t = cpool.tile([128, KT, B], F32, tag="c2")

                    def emit_c2(m, ps, i=i, c2_t=c2_t):
                        nc.scalar.activation(c2_t[:, m, :], ps[:], IDENT,
                                             bias=bz_t[:, i, m:m + 1], scale=2.0)

                    with nc.named_scope(f"pre{i}"):
                        phase(wx_d[i], H, 1024, z_t, emit_c2)

                    h_prev = None
                    with nc.named_scope(f"rec{i}"):
                        for t in range(NS):
                            last = t == NS - 1
                            if last:
                                z_t = apool.tile([128, KT, B], F32R, tag="act")
                                out_t = z_t
                            else:
                                out_t = hpool.tile([128, KT, B], F32, tag="h")
                            for chh in range(2):
                                sl = slice(chh * (KT // 2), (chh + 1) * (KT // 2))
                                if t == 0:
                                    nc.scalar.activation(
                                        out_t[:, sl, :], c2_t[:, sl, :], TANH,
                                        scale=0.5)
                                else:
                                    s_t = spool.tile([128, KT // 2, B], F32,
                                                     tag="s")
                                    nc.vector.tensor_add(
                                        s_t[:], h_prev[:, sl, :], c2_t[:, sl, :])
                                    nc.scalar.activation(
                                        out_t[:, sl, :], s_t[:], TANH, scale=0.5)
                            h_prev = out_t
                else:
                    # dense Wz fallback: c = pre + bz (f32r);
                    # h <- tanh(Wz@h.T + I@c) with Wz streamed from HBM
                    c_t = cpool.tile([128, KT, B], F32R, tag="c2")

                    def emit_c(m, ps, i=i, c_t=c_t):
                        nc.scalar.activation(c_t[:, m, :], ps[:], IDENT,
                                             bias=bz_t[:, i, m:m + 1], scale=1.0)

                    with nc.named_scope(f"pre{i}"):
                        phase(wx_d[i], H, 512, z_t, emit_c)

                    h_prev = None
                    with nc.named_scope(f"rec{i}"):
                        for t in range(NS):
                            out_t = apool.tile([128, KT, B], F32R, tag="act")

                            def emit_h(m, ps, out_t=out_t):
                                nc.scalar.activation(out_t[:, m, :], ps[:], TANH)

                            if t == 0:
                                # h1 = tanh(c)
                                for m in range(KT):
                                    nc.scalar.activation(out_t[:, m, :],
                                                         c_t[:, m, :], TANH)
                            else:
                                def extra(m, ps, c_t=c_t):
                                    nc.tensor.matmul(ps[:], id_t[:], c_t[:, m, :],
                                                     start=True, stop=False)

                                phase(wz_d[i], H, 512, h_prev, emit_h,
                                      extra_mm=extra)
                            h_prev = out_t
                    z_t = h_prev

            # ---- head: out.T = W_head @ z.T + b_head
            o_t = opool.tile([128, OUT // 128, B], F32, tag="ostage")

            def emit_o(m, ps):
                nc.scalar.activation(o_t[:, m, :], ps[:], IDENT,
                                     bias=bh_t[:, m:m + 1], scale=1.0)

            with nc.named_scope("head"):
                phase(whead_d, OUT, 1024, z_t, emit_o)

            nc.sync.dma_start(o_d.rearrange("(t p) n -> p t n", p=128), o_t[:])

    nc.compile()
    return nc


_CACHE = {}


def _get_program(wz_diag: bool):
    if wz_diag not in _CACHE:
        _CACHE[wz_diag] = _build(wz_diag)
    return _CACHE[wz_diag]


def kernel(x, W_in, b_in, Wz, bz, Wx, R, W_head, b_head):
    x = np.asarray(x, dtype=np.float32).reshape(x.shape[0], -1)
    wz_diag = _is_half_identity(np.asarray(Wz))
    nc = _get_program(wz_diag)

    common = {
        "win": _round_f32r(np.asarray(W_in, np.float32).T),
        "whead": _round_f32r(np.asarray(W_head, np.float32).T),
        "bin": np.ascontiguousarray(b_in, np.float32),
        "bh": np.ascontiguousarray(b_head, np.float32),
        "bz": np.ascontiguousarray(
            2.0 * np.asarray(bz, np.float32) if wz_diag else bz, np.float32),
    }
    for i in range(NL):
        common[f"wx{i}"] = _round_f32r(np.asarray(Wx[i], np.float32).T)
    if not wz_diag:
        for i in range(NL):
            common[f"wz{i}"] = _round_f32r(np.asarray(Wz[i], np.float32).T)
        common["ident"] = np.eye(128, dtype=np.float32)

    in_maps = []
    for c in range(NCORES):
        m = dict(common)
        m["xT"] = _round_f32r(x[c * B:(c + 1) * B].T)
        in_maps.append(m)

    res = run_bass_kernel_spmd(nc, in_maps, list(range(NCORES)))
    return np.concatenate(
        [res.results[c]["o"].T for c in range(NCORES)], axis=0)
